# revision 11
# baseline (speedup 1.0000x reference)
"""Trainium2 Bass kernel for nn_DiagLRConv (diag-embedded 5x5 conv, pad=2).

Math: out[n,o,h,w] = sum_{i,k} filter_w[o,i,k] * x[n,i,h+k-2,w+k-2]
(a diag_embed'ed 5x5 kernel is 5 diagonal shifts mixed through 16x16 channel
matrices).

Mapping (per NeuronCore, 2 images each, 8 cores data-parallel over batch):
  - x cast to fp16, zero-padded into flat 517-wide rows (2+512+3 cols) and
    pre-banded on host: each 128-row slab is 2 super-bands of 64 output
    rows.  Super-band u holds [img0 s0 | img1 s0 | img0 s1 | img1 s1] x
    16ch = 64 partitions, where s0 is the raw padded window (69 rows) and
    s1 is its (+1 row, +1 col) diagonal shift.
  - Only s0 is read from HBM (once, ~5 KB/partition descriptors so the DMA
    engines pipeline across partitions).  The s1 halves are generated
    ON-CHIP by VectorE 4x-mode fp16 copies at flat offset +518
    (= +1 row * 517 + 1 col) -- VectorE is otherwise idle.
  - Matmul: tap-paired K=64 tiles: stationary [64,32] applies taps (2p,
    2p+1) to the (s0, s1) halves at once -- 3 rounds instead of 5, and
    block-diagonal N packing computes both images per tile.  8 concurrent
    tiles via tile_position=(64u, 32j): col band j = output row 4s+j of
    super-band u, accumulating into PSUM bank u.  This halves the PE
    instruction count, which is the issue-rate bottleneck (~41 ns/matmul
    on the PE sequencer).
  - PSUM -> SBUF evacuation fp32->fp16 entirely on ScalarE (2 copies per
    step); output DMA per step on the GpSimd SWDGE path so it interleaves
    with sync-ring input at packet granularity.  Host reassembles the
    kernel-native output layout.
"""

import numpy as np

F16 = np.float16

_COMPILED = {}

SBR = 64                  # output rows per super-band per slab
SLAB = 2 * SBR            # 128 output rows per slab
RB = SBR + 5              # 69 buffer rows per super-band
WPAD = 517                # padded row length (2 + 512 + 3)
L = RB * WPAD             # flat fp16 elems per partition per slab
LC = L - WPAD - 1         # s1 copy length (shift source +518)
STEPS = SBR // 4          # 16 steps per slab (4 rows per col band per step)


def _trace_nc(H):
    import concourse.mybir as mybir
    import concourse.tile as tile
    from concourse import bacc

    F32 = mybir.dt.float32
    FP16 = mybir.dt.float16

    assert H % SLAB == 0
    G = H // SLAB

    nc = bacc.Bacc(None, target_bir_lowering=False, debug=False)
    # s0-only banded input: xp[g, 32u+16m+c] = padded rows
    # [128g+64u, +69) x 517 cols of image m, channel c, flattened
    xp = nc.declare_dram_parameter("xp", [G, 64, L], FP16, isOutput=False)
    wd = nc.declare_dram_parameter("wd", [128, 3, 32], FP16, isOutput=False)
    # y[g, s, 32j+16m+o, u, w] = out[m, o, 128g+64u+4s+j, w]
    y = nc.declare_dram_parameter("y", [G, STEPS, 128, 2, 512], FP16, isOutput=True)

    with tile.TileContext(nc) as tc:
        with (
            tc.tile_pool(name="const", bufs=1) as const,
            tc.tile_pool(name="xpool", bufs=2) as xpool,
            tc.tile_pool(name="psum", bufs=8, space="PSUM") as psum,
            tc.tile_pool(name="stpool", bufs=2) as stpool,
        ):
            wt = const.tile([128, 3, 32], FP16)
            nc.sync.dma_start(out=wt[:], in_=wd[:])

            CH = 5 * WPAD    # ~5.2 KB/partition per input descriptor
            CCH = 9 * WPAD   # s1 copy piece (~4.8 us DVE each)
            for g in range(G):
                xq = xpool.tile([128, L], FP16, tag="xq", name=f"xq{g}")
                for c0 in range(0, L, CH):
                    c1 = min(L, c0 + CH)
                    nc.sync.dma_start(out=xq[0:32, c0:c1], in_=xp[g, 0:32, c0:c1])
                    nc.sync.dma_start(out=xq[64:96, c0:c1], in_=xp[g, 32:64, c0:c1])
                # s1 halves: diagonal (+1,+1) shift = flat offset +518
                for d0 in range(0, LC, CCH):
                    d1 = min(LC, d0 + CCH)
                    nc.vector.tensor_copy(
                        xq[32:64, d0:d1], xq[0:32, d0 + 518 : d1 + 518]
                    )
                    nc.vector.tensor_copy(
                        xq[96:128, d0:d1], xq[64:96, d0 + 518 : d1 + 518]
                    )
                for s in range(STEPS):
                    pss = [
                        psum.tile([128, 512], F32, tag="ps", name=f"ps{g}_{s}_{u}")
                        for u in range(2)
                    ]
                    st = stpool.tile([128, 2, 512], FP16, tag="st", name=f"st{g}_{s}")
                    for p in range(3):
                        for u in range(2):
                            for j in range(4):
                                off = (4 * s + j + 2 * p) * WPAD + 2 * p
                                nc.tensor.matmul(
                                    pss[u][32 * j : 32 * j + 32, :],
                                    wt[64 * u : 64 * u + 64, p, :],
                                    xq[64 * u : 64 * u + 64, off : off + 512],
                                    start=(p == 0),
                                    stop=(p == 2),
                                    tile_position=(64 * u, 32 * j),
                                    skip_group_check=True,
                                )
                    for u in range(2):
                        nc.scalar.copy(st[:, u, :], pss[u][:])
                    nc.gpsimd.dma_start(out=y[g, s], in_=st[:])
    nc.compile()
    return nc


def _get_nc(H, **kw):
    key = (H, tuple(sorted(kw.items())))
    if key not in _COMPILED:
        _COMPILED[key] = _trace_nc(H, **kw)
    return _COMPILED[key]


def _prep_inputs(x, filter_w, H):
    """x: [N,16,H,512] fp32, filter_w: [16,16,5] fp32 -> per-core in_maps."""
    N = x.shape[0]
    n_cores = N // 2
    x16 = x.astype(F16)

    wT = np.transpose(filter_w.astype(F16), (1, 2, 0))  # [i, k, o]
    wd = np.zeros((128, 3, 32), dtype=F16)
    for u in range(2):
        b = 64 * u
        for p in range(3):
            wd[b : b + 16, p, 0:16] = wT[:, 2 * p, :]        # img0 s0: tap 2p
            wd[b + 16 : b + 32, p, 16:32] = wT[:, 2 * p, :]  # img1 s0
            if 2 * p + 1 < 5:
                wd[b + 32 : b + 48, p, 0:16] = wT[:, 2 * p + 1, :]    # img0 s1
                wd[b + 48 : b + 64, p, 16:32] = wT[:, 2 * p + 1, :]   # img1 s1

    G = H // SLAB
    row_starts = (
        np.arange(G)[:, None] * SLAB + np.arange(2)[None, :] * SBR
    )  # [G, 2]
    in_maps = []
    for cid in range(n_cores):
        xpf = np.zeros((2, 16, H + 5, WPAD), dtype=F16)
        xpf[:, :, 2 : H + 2, 2:514] = x16[2 * cid : 2 * cid + 2]
        xb = xpf[:, :, row_starts[:, :, None] + np.arange(RB)]  # [2,16,G,2,RB,517]
        xb = np.transpose(xb, (2, 3, 0, 1, 4, 5)).reshape(G, 64, L)
        in_maps.append({"xp": np.ascontiguousarray(xb), "wd": wd})
    return in_maps


def _reassemble(yk, H):
    # yk [G, STEPS, 128, 2, 512]; p = 32j + 16m + o; row = 128g + 64u + 4s + j
    G = H // SLAB
    z = yk.reshape(G, STEPS, 4, 2, 16, 2, 512)      # g, s, j, m, o, u, w
    z = np.transpose(z, (3, 4, 0, 5, 1, 2, 6))      # m, o, g, u, s, j, w
    return z.reshape(2, 16, H, 512).astype(np.float32)


def kernel(x, filter_w):
    from concourse.bass_utils import run_bass_kernel_spmd

    x = np.asarray(x)
    filter_w = np.asarray(filter_w)
    N, C, H, W = x.shape
    assert (C, W) == (16, 512) and N % 2 == 0

    nc = _get_nc(H)
    in_maps = _prep_inputs(x, filter_w, H)
    n_cores = len(in_maps)
    res = run_bass_kernel_spmd(nc, in_maps, list(range(n_cores)))
    out = np.empty((N, 16, H, 512), dtype=np.float32)
    for cid in range(n_cores):
        out[2 * cid : 2 * cid + 2] = _reassemble(res.results[cid]["y"], H)
    return out


if __name__ == "__main__":
    import sys

    H = int(sys.argv[1]) if len(sys.argv) > 1 else 128
    rng = np.random.default_rng(0)
    x = rng.standard_normal((16, 16, H, 512)).astype(np.float32)
    fw = (rng.standard_normal((16, 16, 5)) * 0.1).astype(np.float32)
    out = kernel(x, fw)

    xpad = np.zeros((16, 16, H + 4, 516), dtype=np.float64)
    xpad[:, :, 2 : H + 2, 2:514] = x
    ref = np.zeros_like(out, dtype=np.float64)
    for k in range(5):
        sh = xpad[:, :, k : k + H, k : k + 512]
        ref += np.einsum("oik,nihw->nohw", fw[:, :, k : k + 1].astype(np.float64), sh)
    rel = np.linalg.norm(out - ref) / np.linalg.norm(ref)
    mx = np.abs(out - ref).max() / np.abs(ref).max()
    print(f"self-test H={H}: rel l2 err {rel:.3e}, max err {mx:.3e}")


# revision 12
# speedup vs baseline: 1.2853x; 1.2853x over previous
"""Trainium2 Bass kernel for nn_DiagLRConv (diag-embedded 5x5 conv, pad=2).

Math: out[n,o,h,w] = sum_{i,k} filter_w[o,i,k] * x[n,i,h+k-2,w+k-2]
(a diag_embed'ed 5x5 kernel is 5 diagonal shifts mixed through 16x16 channel
matrices).

Mapping (per NeuronCore, 2 images each, 8 cores data-parallel over batch):
  - x cast to fp16 and zero-padded on host into a flat [2,16,(H+5)*517]
    layout (517 = 2 + 512 + 3 pad columns).  fp16 rounding of x/w is the
    only approximation (~3e-4 rel l2, threshold 2e-2).
  - x is loaded ONCE (no shifted duplicate reads): each 128-row slab is
    4 row-bands of 32 output rows; band i occupies partitions 32i..32i+32
    holding [img0 16ch; img1 16ch] x 37 padded rows x 517 cols, loaded as
    one flat contiguous 38 KB/partition DMA run per (band, image).
  - Diagonal tap k of output row t reads the flat buffer at offset
    (row_in_buf)*517 + k -- no pre-shifted copies needed.
  - Matmul: 16 concurrent 32x32 tiles via tile_position=(32i,32j):
    row-band i = x data band, col-band j = output row t=4s+j.  Stationary
    [K=32,N=32] is block-diagonal: cols 0:16 = img0 out channels, cols
    16:32 = img1, so each tile computes both images at once.  5 tap-rounds
    accumulate into PSUM bank i (4 banks/step, 8 banks double-buffered);
    concurrent tiles on one column strip always target different banks.
  - PSUM -> SBUF evacuation with fp32->fp16 cast, split between ScalarE
    (banks 0,1) and VectorE (banks 2,3); one 512 KB output DMA per step
    in a kernel-native layout; host reassembles.
"""

import numpy as np

F16 = np.float16

_COMPILED = {}

ROWS_PER_BAND = 32            # output rows per row-band per slab
BANDS = 4
SLAB = ROWS_PER_BAND * BANDS  # 128 output rows per slab
RB = ROWS_PER_BAND + 5        # 37 buffer rows per band
WPAD = 517                    # padded row length (2 + 512 + 3)
L = RB * WPAD                 # flat fp16 elems per partition per slab
STEPS = ROWS_PER_BAND // 4    # 8 steps per slab (4 rows per step per band)


def _trace_nc(H):
    import concourse.mybir as mybir
    import concourse.tile as tile
    from concourse import bacc

    F32 = mybir.dt.float32
    FP16 = mybir.dt.float16

    assert H % SLAB == 0
    G = H // SLAB

    nc = bacc.Bacc(None, target_bir_lowering=False, debug=False)
    # banded input layout, host-materialized: xp[g, 32i+16m+c, r*517+w] =
    # xpad[m, c, 128g+32i+r, w] -- so each input DMA spans all 128
    # partitions with ~5 KB/partition descriptors (DMA engines pipeline
    # across partitions only at descriptor granularity).
    xp = nc.declare_dram_parameter("xp", [G, 128, L], FP16, isOutput=False)
    wd = nc.declare_dram_parameter("wd", [128, 5, 32], FP16, isOutput=False)
    # kernel-native output layout; host reassembles:
    # y[g, s, 32j+16m+o, i, w] = out[m, o, 128g+32i+4s+j, w]
    y = nc.declare_dram_parameter("y", [G, STEPS, 128, 4, 512], FP16, isOutput=True)

    with tile.TileContext(nc) as tc:
        with (
            tc.tile_pool(name="const", bufs=1) as const,
            tc.tile_pool(name="xpool", bufs=3) as xpool,
            tc.tile_pool(name="psum", bufs=8, space="PSUM") as psum,
            tc.tile_pool(name="stpool", bufs=2) as stpool,
        ):
            wt = const.tile([128, 5, 32], FP16)
            nc.sync.dma_start(out=wt[:], in_=wd[:])

            CHUNK = 5 * WPAD  # ~5.2 KB/partition per descriptor
            for g in range(G):
                xq = xpool.tile([128, L], FP16, tag="xq", name=f"xq{g}")
                for c0 in range(0, L, CHUNK):
                    c1 = min(L, c0 + CHUNK)
                    nc.sync.dma_start(out=xq[:, c0:c1], in_=xp[g, :, c0:c1])
                for s in range(STEPS):
                    pss = [
                        psum.tile([128, 512], F32, tag="ps", name=f"ps{g}_{s}_{i}")
                        for i in range(BANDS)
                    ]
                    st = stpool.tile([128, 4, 512], FP16, tag="st", name=f"st{g}_{s}")
                    for k in range(5):
                        for i in range(BANDS):
                            for j in range(4):
                                off = (4 * s + j + k) * WPAD + k
                                nc.tensor.matmul(
                                    pss[i][32 * j : 32 * j + 32, :],
                                    wt[32 * i : 32 * i + 32, k, :],
                                    xq[32 * i : 32 * i + 32, off : off + 512],
                                    start=(k == 0),
                                    stop=(k == 4),
                                    tile_position=(32 * i, 32 * j),
                                    skip_group_check=True,
                                )
                    for i in range(BANDS):
                        if i < 2:
                            nc.scalar.copy(st[:, i, :], pss[i][:])
                        else:
                            nc.vector.tensor_copy(st[:, i, :], pss[i][:])
                    # SWDGE (gpsimd) output path: separate descriptor
                    # queues from the sync-ring input stream, so output
                    # transfers interleave with input at packet granularity
                    nc.gpsimd.dma_start(out=y[g, s], in_=st[:])
    nc.compile()
    return nc


def _get_nc(H, **kw):
    key = (H, tuple(sorted(kw.items())))
    if key not in _COMPILED:
        _COMPILED[key] = _trace_nc(H, **kw)
    return _COMPILED[key]


def _prep_inputs(x, filter_w, H):
    """x: [N,16,H,512] fp32, filter_w: [16,16,5] fp32 -> per-core in_maps."""
    N = x.shape[0]
    n_cores = N // 2
    x16 = x.astype(F16)

    wT = np.transpose(filter_w.astype(F16), (1, 2, 0))  # [i, k, o]
    wd = np.zeros((128, 5, 32), dtype=F16)
    for b in range(BANDS):
        wd[32 * b : 32 * b + 16, :, 0:16] = wT
        wd[32 * b + 16 : 32 * b + 32, :, 16:32] = wT

    G = H // SLAB
    row_starts = (
        np.arange(G)[:, None] * SLAB + np.arange(BANDS)[None, :] * ROWS_PER_BAND
    )  # [G, BANDS]
    in_maps = []
    for cid in range(n_cores):
        xpf = np.zeros((2, 16, H + 5, WPAD), dtype=F16)
        xpf[:, :, 2 : H + 2, 2:514] = x16[2 * cid : 2 * cid + 2]
        # banded layout [G, 128, L]: partition 32i+16m+c holds band i's
        # RB padded rows (with halo duplicated across bands)
        xb = xpf[:, :, row_starts[:, :, None] + np.arange(RB)]  # [2,16,G,4,RB,517]
        xb = np.transpose(xb, (2, 3, 0, 1, 4, 5)).reshape(G, 128, L)
        in_maps.append({"xp": np.ascontiguousarray(xb), "wd": wd})
    return in_maps


def _reassemble(yk, H):
    # yk [G, STEPS, 128, 4, 512]; p = 32j + 16m + o; row = 128g + 32i + 4s + j
    G = H // SLAB
    z = yk.reshape(G, STEPS, 4, 2, 16, 4, 512)      # g, s, j, m, o, i, w
    z = np.transpose(z, (3, 4, 0, 5, 1, 2, 6))      # m, o, g, i, s, j, w
    return z.reshape(2, 16, H, 512).astype(np.float32)


def kernel(x, filter_w):
    from concourse.bass_utils import run_bass_kernel_spmd

    x = np.asarray(x)
    filter_w = np.asarray(filter_w)
    N, C, H, W = x.shape
    assert (C, W) == (16, 512) and N % 2 == 0

    nc = _get_nc(H)
    in_maps = _prep_inputs(x, filter_w, H)
    n_cores = len(in_maps)
    res = run_bass_kernel_spmd(nc, in_maps, list(range(n_cores)))
    out = np.empty((N, 16, H, 512), dtype=np.float32)
    for cid in range(n_cores):
        out[2 * cid : 2 * cid + 2] = _reassemble(res.results[cid]["y"], H)
    return out


if __name__ == "__main__":
    import sys

    H = int(sys.argv[1]) if len(sys.argv) > 1 else 128
    rng = np.random.default_rng(0)
    x = rng.standard_normal((16, 16, H, 512)).astype(np.float32)
    fw = (rng.standard_normal((16, 16, 5)) * 0.1).astype(np.float32)
    out = kernel(x, fw)

    xpad = np.zeros((16, 16, H + 4, 516), dtype=np.float64)
    xpad[:, :, 2 : H + 2, 2:514] = x
    ref = np.zeros_like(out, dtype=np.float64)
    for k in range(5):
        sh = xpad[:, :, k : k + H, k : k + 512]
        ref += np.einsum("oik,nihw->nohw", fw[:, :, k : k + 1].astype(np.float64), sh)
    rel = np.linalg.norm(out - ref) / np.linalg.norm(ref)
    mx = np.abs(out - ref).max() / np.abs(ref).max()
    print(f"self-test H={H}: rel l2 err {rel:.3e}, max err {mx:.3e}")


# revision 13
# speedup vs baseline: 1.4155x; 1.1013x over previous
"""Trainium2 Bass kernel for nn_DiagLRConv (diag-embedded 5x5 conv, pad=2).

Math: out[n,o,h,w] = sum_{i,k} filter_w[o,i,k] * x[n,i,h+k-2,w+k-2]
(a diag_embed'ed 5x5 kernel is 5 diagonal shifts mixed through 16x16 channel
matrices).

Mapping (per NeuronCore, 2 images each, 8 cores data-parallel over batch):
  - x cast to fp16 and zero-padded on host into a flat [2,16,(H+5)*517]
    layout (517 = 2 + 512 + 3 pad columns).  fp16 rounding of x/w is the
    only approximation (~3e-4 rel l2, threshold 2e-2).
  - x is loaded ONCE (no shifted duplicate reads): each 128-row slab is
    4 row-bands of 32 output rows; band i occupies partitions 32i..32i+32
    holding [img0 16ch; img1 16ch] x 37 padded rows x 517 cols, loaded as
    one flat contiguous 38 KB/partition DMA run per (band, image).
  - Diagonal tap k of output row t reads the flat buffer at offset
    (row_in_buf)*517 + k -- no pre-shifted copies needed.
  - Matmul: 16 concurrent 32x32 tiles via tile_position=(32i,32j):
    row-band i = x data band, col-band j = output row t=4s+j.  Stationary
    [K=32,N=32] is block-diagonal: cols 0:16 = img0 out channels, cols
    16:32 = img1, so each tile computes both images at once.  5 tap-rounds
    accumulate into PSUM bank i (4 banks/step, 8 banks double-buffered);
    concurrent tiles on one column strip always target different banks.
  - PSUM -> SBUF evacuation with fp32->fp16 cast, split between ScalarE
    (banks 0,1) and VectorE (banks 2,3); one 512 KB output DMA per step
    in a kernel-native layout; host reassembles.
"""

import numpy as np

F16 = np.float16

_COMPILED = {}

ROWS_PER_BAND = 32            # output rows per row-band per slab
BANDS = 4
SLAB = ROWS_PER_BAND * BANDS  # 128 output rows per slab
RB = ROWS_PER_BAND + 5        # 37 buffer rows per band
WPAD = 517                    # padded row length (2 + 512 + 3)
L = RB * WPAD                 # flat fp16 elems per partition per slab
STEPS = ROWS_PER_BAND // 4    # 8 steps per slab (4 rows per step per band)


def _trace_nc(H):
    import concourse.mybir as mybir
    import concourse.tile as tile
    from concourse import bacc

    F32 = mybir.dt.float32
    FP16 = mybir.dt.float16

    assert H % SLAB == 0
    G = H // SLAB

    nc = bacc.Bacc(None, target_bir_lowering=False, debug=False)
    # banded input layout, host-materialized: xp[g, 32i+16m+c, r*517+w] =
    # xpad[m, c, 128g+32i+r, w] -- so each input DMA spans all 128
    # partitions with ~5 KB/partition descriptors (DMA engines pipeline
    # across partitions only at descriptor granularity).
    xp = nc.declare_dram_parameter("xp", [G, 128, L], FP16, isOutput=False)
    wd = nc.declare_dram_parameter("wd", [128, 5, 32], FP16, isOutput=False)
    # kernel-native output layout; host reassembles:
    # y[g, s, 32j+16m+o, i, w] = out[m, o, 128g+32i+4s+j, w]
    y = nc.declare_dram_parameter("y", [G, STEPS, 128, 4, 512], FP16, isOutput=True)

    with tile.TileContext(nc) as tc:
        with (
            tc.tile_pool(name="const", bufs=1) as const,
            tc.tile_pool(name="xpool", bufs=3) as xpool,
            tc.tile_pool(name="psum", bufs=8, space="PSUM") as psum,
            tc.tile_pool(name="stpool", bufs=4) as stpool,
        ):
            wt = const.tile([128, 5, 32], FP16)
            nc.sync.dma_start(out=wt[:], in_=wd[:])

            CHUNK = 5 * WPAD  # ~5.2 KB/partition per descriptor
            for g in range(G):
                xq = xpool.tile([128, L], FP16, tag="xq", name=f"xq{g}")
                for c0 in range(0, L, CHUNK):
                    c1 = min(L, c0 + CHUNK)
                    nc.sync.dma_start(out=xq[:, c0:c1], in_=xp[g, :, c0:c1])
                for s in range(STEPS):
                    pss = [
                        psum.tile([128, 512], F32, tag="ps", name=f"ps{g}_{s}_{i}")
                        for i in range(BANDS)
                    ]
                    st = stpool.tile([128, 4, 512], FP16, tag="st", name=f"st{g}_{s}")
                    for k in range(5):
                        for i in range(BANDS):
                            for j in range(4):
                                off = (4 * s + j + k) * WPAD + k
                                nc.tensor.matmul(
                                    pss[i][32 * j : 32 * j + 32, :],
                                    wt[32 * i : 32 * i + 32, k, :],
                                    xq[32 * i : 32 * i + 32, off : off + 512],
                                    start=(k == 0),
                                    stop=(k == 4),
                                    tile_position=(32 * i, 32 * j),
                                    skip_group_check=True,
                                )
                    for i in range(BANDS):
                        if i < 2:
                            nc.scalar.copy(st[:, i, :], pss[i][:])
                        else:
                            nc.vector.tensor_copy(st[:, i, :], pss[i][:])
                    # SWDGE (gpsimd) output path: separate descriptor
                    # queues from the sync-ring input stream, so output
                    # transfers interleave with input at packet granularity
                    nc.gpsimd.dma_start(out=y[g, s], in_=st[:])
    nc.compile()
    return nc


def _get_nc(H, **kw):
    key = (H, tuple(sorted(kw.items())))
    if key not in _COMPILED:
        _COMPILED[key] = _trace_nc(H, **kw)
    return _COMPILED[key]


def _prep_inputs(x, filter_w, H):
    """x: [N,16,H,512] fp32, filter_w: [16,16,5] fp32 -> per-core in_maps."""
    N = x.shape[0]
    n_cores = N // 2
    x16 = x.astype(F16)

    wT = np.transpose(filter_w.astype(F16), (1, 2, 0))  # [i, k, o]
    wd = np.zeros((128, 5, 32), dtype=F16)
    for b in range(BANDS):
        wd[32 * b : 32 * b + 16, :, 0:16] = wT
        wd[32 * b + 16 : 32 * b + 32, :, 16:32] = wT

    G = H // SLAB
    row_starts = (
        np.arange(G)[:, None] * SLAB + np.arange(BANDS)[None, :] * ROWS_PER_BAND
    )  # [G, BANDS]
    in_maps = []
    for cid in range(n_cores):
        xpf = np.zeros((2, 16, H + 5, WPAD), dtype=F16)
        xpf[:, :, 2 : H + 2, 2:514] = x16[2 * cid : 2 * cid + 2]
        # banded layout [G, 128, L]: partition 32i+16m+c holds band i's
        # RB padded rows (with halo duplicated across bands)
        xb = xpf[:, :, row_starts[:, :, None] + np.arange(RB)]  # [2,16,G,4,RB,517]
        xb = np.transpose(xb, (2, 3, 0, 1, 4, 5)).reshape(G, 128, L)
        in_maps.append({"xp": np.ascontiguousarray(xb), "wd": wd})
    return in_maps


def _reassemble(yk, H):
    # yk [G, STEPS, 128, 4, 512]; p = 32j + 16m + o; row = 128g + 32i + 4s + j
    G = H // SLAB
    z = yk.reshape(G, STEPS, 4, 2, 16, 4, 512)      # g, s, j, m, o, i, w
    z = np.transpose(z, (3, 4, 0, 5, 1, 2, 6))      # m, o, g, i, s, j, w
    return z.reshape(2, 16, H, 512).astype(np.float32)


def kernel(x, filter_w):
    from concourse.bass_utils import run_bass_kernel_spmd

    x = np.asarray(x)
    filter_w = np.asarray(filter_w)
    N, C, H, W = x.shape
    assert (C, W) == (16, 512) and N % 2 == 0

    nc = _get_nc(H)
    in_maps = _prep_inputs(x, filter_w, H)
    n_cores = len(in_maps)
    res = run_bass_kernel_spmd(nc, in_maps, list(range(n_cores)))
    out = np.empty((N, 16, H, 512), dtype=np.float32)
    for cid in range(n_cores):
        out[2 * cid : 2 * cid + 2] = _reassemble(res.results[cid]["y"], H)
    return out


if __name__ == "__main__":
    import sys

    H = int(sys.argv[1]) if len(sys.argv) > 1 else 128
    rng = np.random.default_rng(0)
    x = rng.standard_normal((16, 16, H, 512)).astype(np.float32)
    fw = (rng.standard_normal((16, 16, 5)) * 0.1).astype(np.float32)
    out = kernel(x, fw)

    xpad = np.zeros((16, 16, H + 4, 516), dtype=np.float64)
    xpad[:, :, 2 : H + 2, 2:514] = x
    ref = np.zeros_like(out, dtype=np.float64)
    for k in range(5):
        sh = xpad[:, :, k : k + H, k : k + 512]
        ref += np.einsum("oik,nihw->nohw", fw[:, :, k : k + 1].astype(np.float64), sh)
    rel = np.linalg.norm(out - ref) / np.linalg.norm(ref)
    mx = np.abs(out - ref).max() / np.abs(ref).max()
    print(f"self-test H={H}: rel l2 err {rel:.3e}, max err {mx:.3e}")


# revision 14
# speedup vs baseline: 1.4460x; 1.0215x over previous
"""Trainium2 Bass kernel for nn_DiagLRConv (diag-embedded 5x5 conv, pad=2).

Math: out[n,o,h,w] = sum_{i,k} filter_w[o,i,k] * x[n,i,h+k-2,w+k-2]
(a diag_embed'ed 5x5 kernel is 5 diagonal shifts mixed through 16x16 channel
matrices).

Mapping (per NeuronCore, 2 images each, 8 cores data-parallel over batch):
  - x cast to fp16 and zero-padded on host into a flat [2,16,(H+5)*517]
    layout (517 = 2 + 512 + 3 pad columns).  fp16 rounding of x/w is the
    only approximation (~3e-4 rel l2, threshold 2e-2).
  - x is loaded ONCE (no shifted duplicate reads): each 128-row slab is
    4 row-bands of 32 output rows; band i occupies partitions 32i..32i+32
    holding [img0 16ch; img1 16ch] x 37 padded rows x 517 cols, loaded as
    one flat contiguous 38 KB/partition DMA run per (band, image).
  - Diagonal tap k of output row t reads the flat buffer at offset
    (row_in_buf)*517 + k -- no pre-shifted copies needed.
  - Matmul: 16 concurrent 32x32 tiles via tile_position=(32i,32j):
    row-band i = x data band, col-band j = output row t=4s+j.  Stationary
    [K=32,N=32] is block-diagonal: cols 0:16 = img0 out channels, cols
    16:32 = img1, so each tile computes both images at once.  5 tap-rounds
    accumulate into PSUM bank i (4 banks/step, 8 banks double-buffered);
    concurrent tiles on one column strip always target different banks.
  - PSUM -> SBUF evacuation with fp32->fp16 cast, split between ScalarE
    (banks 0,1) and VectorE (banks 2,3); one 512 KB output DMA per step
    in a kernel-native layout; host reassembles.
"""

import numpy as np

F16 = np.float16

_COMPILED = {}

ROWS_PER_BAND = 32            # output rows per row-band per slab
BANDS = 4
SLAB = ROWS_PER_BAND * BANDS  # 128 output rows per slab
RB = ROWS_PER_BAND + 5        # 37 buffer rows per band
WPAD = 517                    # padded row length (2 + 512 + 3)
L = RB * WPAD                 # flat fp16 elems per partition per slab
STEPS = ROWS_PER_BAND // 4    # 8 steps per slab (4 rows per step per band)


def _trace_nc(H):
    import concourse.mybir as mybir
    import concourse.tile as tile
    from concourse import bacc

    F32 = mybir.dt.float32
    FP16 = mybir.dt.float16

    assert H % SLAB == 0
    G = H // SLAB

    nc = bacc.Bacc(None, target_bir_lowering=False, debug=False)
    # banded input layout, host-materialized: xp[g, 32i+16m+c, r*517+w] =
    # xpad[m, c, 128g+32i+r, w] -- so each input DMA spans all 128
    # partitions with ~5 KB/partition descriptors (DMA engines pipeline
    # across partitions only at descriptor granularity).
    xp = nc.declare_dram_parameter("xp", [G, 128, L], FP16, isOutput=False)
    wd = nc.declare_dram_parameter("wd", [128, 5, 32], FP16, isOutput=False)
    # kernel-native output layout; host reassembles:
    # y[g, s, 32j+16m+o, i, w] = out[m, o, 128g+32i+4s+j, w]
    y = nc.declare_dram_parameter("y", [G, STEPS, 128, 4, 512], FP16, isOutput=True)

    with tile.TileContext(nc) as tc:
        with (
            tc.tile_pool(name="const", bufs=1) as const,
            tc.tile_pool(name="xpool", bufs=3) as xpool,
            tc.tile_pool(name="psum", bufs=4, space="PSUM") as psum,
            tc.tile_pool(name="stpool", bufs=4) as stpool,
        ):
            wt = const.tile([128, 5, 32], FP16)
            nc.sync.dma_start(out=wt[:], in_=wd[:])

            CHUNK = 5 * WPAD  # ~5.2 KB/partition per descriptor
            for g in range(G):
                xq = xpool.tile([128, L], FP16, tag="xq", name=f"xq{g}")
                for c0 in range(0, L, CHUNK):
                    c1 = min(L, c0 + CHUNK)
                    nc.sync.dma_start(out=xq[:, c0:c1], in_=xp[g, :, c0:c1])
                for s in range(STEPS):
                    # two 2-bank PSUM tiles per step: band i -> pair i//2,
                    # bank i%2; evac is one 1024-col copy per engine
                    pss = [
                        psum.tile([128, 2, 512], F32, tag="ps", name=f"ps{g}_{s}_{p}")
                        for p in range(2)
                    ]
                    st = stpool.tile([128, 4, 512], FP16, tag="st", name=f"st{g}_{s}")
                    for k in range(5):
                        for i in range(BANDS):
                            for j in range(4):
                                off = (4 * s + j + k) * WPAD + k
                                nc.tensor.matmul(
                                    pss[i // 2][32 * j : 32 * j + 32, i % 2, :],
                                    wt[32 * i : 32 * i + 32, k, :],
                                    xq[32 * i : 32 * i + 32, off : off + 512],
                                    start=(k == 0),
                                    stop=(k == 4),
                                    tile_position=(32 * i, 32 * j),
                                    skip_group_check=True,
                                )
                    nc.scalar.copy(st[:, 0:2, :], pss[0][:])
                    nc.vector.tensor_copy(st[:, 2:4, :], pss[1][:])
                    # SWDGE (gpsimd) output path: separate descriptor
                    # queues from the sync-ring input stream, so output
                    # transfers interleave with input at packet granularity
                    nc.gpsimd.dma_start(out=y[g, s], in_=st[:])
    nc.compile()
    return nc


def _get_nc(H, **kw):
    key = (H, tuple(sorted(kw.items())))
    if key not in _COMPILED:
        _COMPILED[key] = _trace_nc(H, **kw)
    return _COMPILED[key]


def _prep_inputs(x, filter_w, H):
    """x: [N,16,H,512] fp32, filter_w: [16,16,5] fp32 -> per-core in_maps."""
    N = x.shape[0]
    n_cores = N // 2
    x16 = x.astype(F16)

    wT = np.transpose(filter_w.astype(F16), (1, 2, 0))  # [i, k, o]
    wd = np.zeros((128, 5, 32), dtype=F16)
    for b in range(BANDS):
        wd[32 * b : 32 * b + 16, :, 0:16] = wT
        wd[32 * b + 16 : 32 * b + 32, :, 16:32] = wT

    G = H // SLAB
    row_starts = (
        np.arange(G)[:, None] * SLAB + np.arange(BANDS)[None, :] * ROWS_PER_BAND
    )  # [G, BANDS]
    in_maps = []
    for cid in range(n_cores):
        xpf = np.zeros((2, 16, H + 5, WPAD), dtype=F16)
        xpf[:, :, 2 : H + 2, 2:514] = x16[2 * cid : 2 * cid + 2]
        # banded layout [G, 128, L]: partition 32i+16m+c holds band i's
        # RB padded rows (with halo duplicated across bands)
        xb = xpf[:, :, row_starts[:, :, None] + np.arange(RB)]  # [2,16,G,4,RB,517]
        xb = np.transpose(xb, (2, 3, 0, 1, 4, 5)).reshape(G, 128, L)
        in_maps.append({"xp": np.ascontiguousarray(xb), "wd": wd})
    return in_maps


def _reassemble(yk, H):
    # yk [G, STEPS, 128, 4, 512]; p = 32j + 16m + o; row = 128g + 32i + 4s + j
    G = H // SLAB
    z = yk.reshape(G, STEPS, 4, 2, 16, 4, 512)      # g, s, j, m, o, i, w
    z = np.transpose(z, (3, 4, 0, 5, 1, 2, 6))      # m, o, g, i, s, j, w
    return z.reshape(2, 16, H, 512).astype(np.float32)


def kernel(x, filter_w):
    from concourse.bass_utils import run_bass_kernel_spmd

    x = np.asarray(x)
    filter_w = np.asarray(filter_w)
    N, C, H, W = x.shape
    assert (C, W) == (16, 512) and N % 2 == 0

    nc = _get_nc(H)
    in_maps = _prep_inputs(x, filter_w, H)
    n_cores = len(in_maps)
    res = run_bass_kernel_spmd(nc, in_maps, list(range(n_cores)))
    out = np.empty((N, 16, H, 512), dtype=np.float32)
    for cid in range(n_cores):
        out[2 * cid : 2 * cid + 2] = _reassemble(res.results[cid]["y"], H)
    return out


if __name__ == "__main__":
    import sys

    H = int(sys.argv[1]) if len(sys.argv) > 1 else 128
    rng = np.random.default_rng(0)
    x = rng.standard_normal((16, 16, H, 512)).astype(np.float32)
    fw = (rng.standard_normal((16, 16, 5)) * 0.1).astype(np.float32)
    out = kernel(x, fw)

    xpad = np.zeros((16, 16, H + 4, 516), dtype=np.float64)
    xpad[:, :, 2 : H + 2, 2:514] = x
    ref = np.zeros_like(out, dtype=np.float64)
    for k in range(5):
        sh = xpad[:, :, k : k + H, k : k + 512]
        ref += np.einsum("oik,nihw->nohw", fw[:, :, k : k + 1].astype(np.float64), sh)
    rel = np.linalg.norm(out - ref) / np.linalg.norm(ref)
    mx = np.abs(out - ref).max() / np.abs(ref).max()
    print(f"self-test H={H}: rel l2 err {rel:.3e}, max err {mx:.3e}")
